# revision 4
# baseline (speedup 1.0000x reference)
"""AttentionHead kernel for 8 Trainium2 NeuronCores (SPMD data-parallel).

Problem: q/k/v projections [1024->64] + masked softmax attention,
B=4, S=2048, d_model=1024, d_k=64.

Sharding: 8 cores = 4 batches x 2 query-halves. Each core handles one
(batch, q-half): query shard [1024, 1024], full key/value for its batch
[2048, 1024], mask shard [1024, 2048]. Weights replicated.

Per-core device layout (everything contracts on the partition dim):
  - inputs are host-transposed to [d_model, seq] bf16 so projections run
    as matmul(out=[64, s], lhsT=w_t[m_blk, 64], rhs=xT[m_blk, s_chunk])
  - v is projected to natural [skv, 64] layout (lhsT=valueT block) and
    augmented with a ones column -> PV matmul yields the softmax
    denominator as a free 65th output column
  - scores are computed TRANSPOSED [skv_tile=128, sq=1024] so no
    probability transpose is ever needed
  - mask (uint8, 1=masked) zeroes scores in PSUM via copy_predicated;
    exp(0)=1 reproduces the reference's where(mask, 1e-9, s) -> exp
  - ACT exp applies the 1024**-0.5 scale for free
"""

import numpy as np
import ml_dtypes

B = 4
S = 2048
D_MODEL = 1024
D_K = 64
N_CORES = 8

P = 128
SQ = S // 2          # per-core query rows (1024)
SKV = S              # per-core kv rows (2048)
MB = D_MODEL // P    # 8 m-blocks (contraction)
JT = SKV // P        # 16 skv tiles
IT = SQ // P         # 8 sq tiles
NQC = SQ // 512      # 2 q-projection chunks
NKC = SKV // 512     # 4 k-projection chunks

_BF16 = ml_dtypes.bfloat16

_cached_nc = None


def _build_nc():
    import concourse.mybir as mybir
    import concourse.tile as tile
    from concourse import bacc

    bf16 = mybir.dt.bfloat16
    f32 = mybir.dt.float32
    u8 = mybir.dt.uint8

    nc = bacc.Bacc(None, target_bir_lowering=False)

    wq_d = nc.dram_tensor("wq_t", [D_MODEL, D_K], bf16, kind="ExternalInput")
    wk_d = nc.dram_tensor("wk_t", [D_MODEL, D_K], bf16, kind="ExternalInput")
    wv_d = nc.dram_tensor("wv_t", [D_MODEL, D_K], bf16, kind="ExternalInput")
    q_d = nc.dram_tensor("q_t", [D_MODEL, SQ], bf16, kind="ExternalInput")
    k_d = nc.dram_tensor("k_t", [D_MODEL, SKV], bf16, kind="ExternalInput")
    v_d = nc.dram_tensor("v_t", [D_MODEL, SKV], bf16, kind="ExternalInput")
    m_d = nc.dram_tensor("mask_t", [SKV, SQ], u8, kind="ExternalInput")
    out_d = nc.dram_tensor("out", [SQ, D_K], f32, kind="ExternalOutput")

    with tile.TileContext(nc) as tc:
        with (
            tc.tile_pool(name="const", bufs=1) as cpool,
            tc.tile_pool(name="inp", bufs=1) as ipool,
            tc.tile_pool(name="proj", bufs=1) as jpool,
            tc.tile_pool(name="fin", bufs=2) as fpool,
            tc.tile_pool(name="ps_proj", bufs=1, space="PSUM") as ps_proj,
            tc.tile_pool(name="ps_pv", bufs=1, space="PSUM") as ps_pv,
            tc.tile_pool(name="ps_s", bufs=2, space="PSUM") as ps_s,
            tc.tile_pool(name="ps_o", bufs=1, space="PSUM") as ps_o,
        ):
            # ---- DMA: weights, then queryT, keyT ----
            wq_sb = cpool.tile([P, MB, D_K], bf16, tag="wq")
            wk_sb = cpool.tile([P, MB, D_K], bf16, tag="wk")
            wv_sb = cpool.tile([P, MB, D_K], bf16, tag="wv")
            nc.sync.dma_start(out=wq_sb, in_=wq_d.rearrange("(mb p) k -> p mb k", p=P))
            nc.sync.dma_start(out=wk_sb, in_=wk_d.rearrange("(mb p) k -> p mb k", p=P))
            nc.sync.dma_start(out=wv_sb, in_=wv_d.rearrange("(mb p) k -> p mb k", p=P))

            qts = []
            for i in range(MB):
                t = ipool.tile([P, SQ], bf16, tag=f"q{i}")
                nc.sync.dma_start(out=t, in_=q_d[i * P : (i + 1) * P, :])
                qts.append(t)
            kts = []
            for i in range(MB):
                t = ipool.tile([P, SKV], bf16, tag=f"k{i}")
                nc.sync.dma_start(out=t, in_=k_d[i * P : (i + 1) * P, :])
                kts.append(t)

            # valueT column-blocks [128, mb, 128] (all m for one skv block),
            # interleaved with mask tiles so both trickle in during the
            # scores pipeline.
            v_r = v_d.rearrange("(mb p) s -> p mb s", p=P)
            vbs = []
            mts = []
            for j in range(JT):
                vb = ipool.tile([P, MB, P], bf16, tag=f"v{j}")
                nc.sync.dma_start(out=vb, in_=v_r[:, :, j * P : (j + 1) * P])
                vbs.append(vb)
                mt = ipool.tile([P, SQ], u8, tag=f"m{j}")
                nc.sync.dma_start(out=mt, in_=m_d[j * P : (j + 1) * P, :])
                mts.append(mt)

            # ---- constants ----
            zeros = cpool.tile([P, SQ], f32, tag="zeros")
            nc.vector.memset(zeros, 0.0)

            # ---- q/k projections -> qT [64, SQ], kT [64, SKV] (bf16) ----
            qT_sb = jpool.tile([D_K, SQ], bf16, tag="qT")
            kT_sb = jpool.tile([D_K, SKV], bf16, tag="kT")
            for t in range(NQC):
                pp = ps_proj.tile([D_K, 512], f32, tag="pqk")
                for i in range(MB):
                    nc.tensor.matmul(
                        pp,
                        lhsT=wq_sb[:, i, :],
                        rhs=qts[i][:, t * 512 : (t + 1) * 512],
                        start=(i == 0),
                        stop=(i == MB - 1),
                    )
                nc.vector.tensor_copy(qT_sb[:, t * 512 : (t + 1) * 512], pp)
            for t in range(NKC):
                pp = ps_proj.tile([D_K, 512], f32, tag="pqk")
                for i in range(MB):
                    nc.tensor.matmul(
                        pp,
                        lhsT=wk_sb[:, i, :],
                        rhs=kts[i][:, t * 512 : (t + 1) * 512],
                        start=(i == 0),
                        stop=(i == MB - 1),
                    )
                nc.vector.tensor_copy(kT_sb[:, t * 512 : (t + 1) * 512], pp)

            # ---- v-aug [128, 16, 65] bf16 (col 64 = ones) ----
            vaug = jpool.tile([P, JT, D_K + 1], bf16, tag="vaug")
            nc.vector.memset(vaug[:, :, D_K : D_K + 1], 1.0)

            # ---- E tiles [128, 16, 1024] bf16 ----
            E = jpool.tile([P, JT, SQ], bf16, tag="E")

            # ---- output PSUM, transposed: [65, sq=1024] f32 = 2 banks.
            # One accumulation group per bank (512-col chunk) — PSUM allows
            # only one live group per 2KB zero region.
            oTp = ps_o.tile([D_K + 1, SQ], f32, tag="oT")

            # identity for the final PE transpose
            idn = cpool.tile([D_K + 1, D_K + 1], f32, tag="idn")
            from concourse.masks import make_identity

            make_identity(nc, idn)

            # ---- main pipeline over skv tiles ----
            for j in range(JT):
                # v projection for this skv block: [128, 64]
                pv = ps_pv.tile([P, D_K], f32, tag="pv")
                for i in range(MB):
                    nc.tensor.matmul(
                        pv,
                        lhsT=vbs[j][:, i, :],
                        rhs=wv_sb[:, i, :],
                        start=(i == 0),
                        stop=(i == MB - 1),
                    )
                nc.vector.tensor_copy(vaug[:, j, 0:D_K], pv)

                # transposed scores [skv_tile 128, sq 1024] (unscaled)
                sp = ps_s.tile([P, SQ], f32, tag="sp")
                for c in range(NQC):
                    nc.tensor.matmul(
                        sp[:, c * 512 : (c + 1) * 512],
                        lhsT=kT_sb[:, j * P : (j + 1) * P],
                        rhs=qT_sb[:, c * 512 : (c + 1) * 512],
                        start=True,
                        stop=True,
                    )
                # mask: zero masked scores in psum (exp(0)=1 == exp(1e-9))
                nc.vector.copy_predicated(out=sp, mask=mts[j], data=zeros)
                # E = exp(s / sqrt(d_model)), cast to bf16
                nc.scalar.activation(
                    out=E[:, j, :],
                    in_=sp,
                    func=mybir.ActivationFunctionType.Exp,
                    scale=float(D_MODEL) ** -0.5,
                )
                # PV accumulation (transposed): oT[65, sq] += vaug_j.T @ E_j
                for c in range(NQC):
                    nc.tensor.matmul(
                        oTp[:, c * 512 : (c + 1) * 512],
                        lhsT=vaug[:, j, :],
                        rhs=E[:, j, c * 512 : (c + 1) * 512],
                        start=(j == 0),
                        stop=(j == JT - 1),
                    )

            # ---- finalize: transpose oT back, divide by ones-row, DMA ----
            oT_sb = jpool.tile([D_K + 1, SQ], f32, tag="oTs")
            nc.vector.tensor_copy(oT_sb, oTp)
            for i in range(IT):
                tp = ps_pv.tile([P, D_K + 1], f32, tag="pv")
                nc.tensor.transpose(
                    tp, in_=oT_sb[:, i * P : (i + 1) * P], identity=idn
                )
                r = fpool.tile([P, 1], f32, tag="r")
                nc.vector.reciprocal(r, tp[:, D_K : D_K + 1])
                ob = fpool.tile([P, D_K], f32, tag="ob")
                nc.vector.tensor_scalar_mul(ob, tp[:, 0:D_K], r)
                nc.sync.dma_start(out=out_d[i * P : (i + 1) * P, :], in_=ob)

    nc.finalize()
    return nc


def _get_nc():
    global _cached_nc
    if _cached_nc is None:
        _cached_nc = _build_nc()
    return _cached_nc


def _shard_inputs(query, key, value, mask, w_q, w_k, w_v):
    """Host-side shard + layout prep. Core c -> (batch c//2, q-half c%2)."""
    wq_t = np.ascontiguousarray(w_q.T).astype(_BF16)
    wk_t = np.ascontiguousarray(w_k.T).astype(_BF16)
    wv_t = np.ascontiguousarray(w_v.T).astype(_BF16)
    in_maps = []
    for c in range(N_CORES):
        b, h = divmod(c, 2)
        s0 = h * SQ
        in_maps.append(
            {
                "wq_t": wq_t,
                "wk_t": wk_t,
                "wv_t": wv_t,
                "q_t": query[b, s0 : s0 + SQ, :].T.astype(_BF16),
                "k_t": key[b].T.astype(_BF16),
                "v_t": value[b].T.astype(_BF16),
                "mask_t": mask[b, s0 : s0 + SQ, :].T.astype(np.uint8),
            }
        )
    return in_maps


def run(inputs, trace=False):
    """Run the SPMD kernel; returns (output [B,S,D_K] f32, BassKernelResults)."""
    from concourse.bass_utils import run_bass_kernel_spmd

    nc = _get_nc()
    in_maps = _shard_inputs(**inputs)
    res = run_bass_kernel_spmd(
        nc, in_maps, core_ids=list(range(N_CORES)), trace=trace
    )
    out = np.empty((B, S, D_K), np.float32)
    for c in range(N_CORES):
        b, h = divmod(c, 2)
        out[b, h * SQ : (h + 1) * SQ, :] = res.results[c]["out"]
    return out, res


def kernel(**inputs):
    out, _ = run(inputs, trace=False)
    return out


# revision 5
# speedup vs baseline: 1.3022x; 1.3022x over previous
"""AttentionHead kernel for 8 Trainium2 NeuronCores (SPMD data-parallel).

Problem: q/k/v projections [1024->64] + masked softmax attention,
B=4, S=2048, d_model=1024, d_k=64.

Sharding: 8 cores = 4 batches x 2 query-halves. Each core handles one
(batch, q-half): query shard [1024, 1024], full key/value for its batch
[2048, 1024], mask shard [1024, 2048]. Weights replicated.

Per-core device pipeline (everything contracts on the partition dim;
all inputs host-packed so each DMA is one large contiguous transfer):
  - projections: qT [64, sq], kT [64, skv] via matmul(lhsT=w_t, rhs=xT)
  - v projected per skv-block to natural [128, 64] (lhsT=valueT block),
    augmented with a ones column
  - scores computed TRANSPOSED [skv_tile=128, sq=1024]: no probability
    transpose needed anywhere
  - ACT exp reads scores straight from PSUM with the 1024**-0.5 scale
    fused; masked positions then set to 1.0 (==exp(1e-9) in fp32) via
    copy_predicated on the bf16 E tile
  - PV accumulates transposed: oT[65, sq] += vaug_j.T @ E_j; row 64 is
    the softmax denominator (free via the ones column)
  - finalize: PE-transpose oT back to [sq, 65], multiply by reciprocal
    of column 64, one batched output DMA
"""

import numpy as np
import ml_dtypes

B = 4
S = 2048
D_MODEL = 1024
D_K = 64
N_CORES = 8

P = 128
SQ = S // 2          # per-core query rows (1024)
SKV = S              # per-core kv rows (2048)
MB = D_MODEL // P    # 8 m-blocks (contraction)
JT = SKV // P        # 16 skv tiles
IT = SQ // P         # 8 sq tiles
NQC = SQ // 512      # 2 q chunks
NKC = SKV // 512     # 4 k chunks
VG = 4               # v/mask DMA groups
JPG = JT // VG       # skv tiles per DMA group

_BF16 = ml_dtypes.bfloat16

_cached_nc = None


def _build_nc():
    import concourse.mybir as mybir
    import concourse.tile as tile
    from concourse import bacc
    from concourse.masks import make_identity

    bf16 = mybir.dt.bfloat16
    f32 = mybir.dt.float32
    u8 = mybir.dt.uint8

    nc = bacc.Bacc(None, target_bir_lowering=False)

    w_d = nc.dram_tensor("w_all", [P, MB, 3 * D_K], bf16, kind="ExternalInput")
    q_d = nc.dram_tensor("q_t", [P, MB, SQ], bf16, kind="ExternalInput")
    k_d = nc.dram_tensor("k_t", [P, MB, SKV], bf16, kind="ExternalInput")
    v_d = nc.dram_tensor("v_t", [P, JT, MB, P], bf16, kind="ExternalInput")
    m_d = nc.dram_tensor("mask_t", [P, JT, SQ], u8, kind="ExternalInput")
    out_d = nc.dram_tensor("out", [P, IT, D_K], f32, kind="ExternalOutput")

    with tile.TileContext(nc) as tc:
        with (
            tc.tile_pool(name="const", bufs=1) as cpool,
            tc.tile_pool(name="inp", bufs=1) as ipool,
            tc.tile_pool(name="proj", bufs=1) as jpool,
            tc.tile_pool(name="fin", bufs=2) as fpool,
            tc.tile_pool(name="ps_proj", bufs=1, space="PSUM") as ps_proj,
            tc.tile_pool(name="ps_pv", bufs=1, space="PSUM") as ps_pv,
            tc.tile_pool(name="ps_s", bufs=2, space="PSUM") as ps_s,
            tc.tile_pool(name="ps_o", bufs=1, space="PSUM") as ps_o,
        ):
            # ---- input DMAs: few, large, contiguous ----
            w_sb = cpool.tile([P, MB, 3 * D_K], bf16, tag="w")
            nc.sync.dma_start(out=w_sb, in_=w_d[:])
            q_sb = ipool.tile([P, MB, SQ], bf16, tag="q")
            nc.sync.dma_start(out=q_sb, in_=q_d[:])
            k_sb = ipool.tile([P, MB, SKV], bf16, tag="k")
            nc.sync.dma_start(out=k_sb, in_=k_d[:])

            # v and mask trickle in groups, mask group ahead of v group
            mgs = []
            vgs = []
            for g in range(VG):
                mg = ipool.tile([P, JPG, SQ], u8, tag=f"m{g}")
                nc.sync.dma_start(out=mg, in_=m_d[:, g * JPG : (g + 1) * JPG, :])
                mgs.append(mg)
                vg = ipool.tile([P, JPG, MB, P], bf16, tag=f"v{g}")
                nc.sync.dma_start(out=vg, in_=v_d[:, g * JPG : (g + 1) * JPG, :, :])
                vgs.append(vg)

            def wq(i):
                return w_sb[:, i, 0:D_K]

            def wk(i):
                return w_sb[:, i, D_K : 2 * D_K]

            def wv(i):
                return w_sb[:, i, 2 * D_K : 3 * D_K]

            # ---- constants ----
            ones_bf = cpool.tile([P, SQ], bf16, tag="ones")
            nc.vector.memset(ones_bf, 1.0)
            idn = cpool.tile([D_K + 1, D_K + 1], f32, tag="idn")
            make_identity(nc, idn)

            # ---- q/k projections -> qT [64, SQ], kT [64, SKV] (bf16) ----
            qT_sb = jpool.tile([D_K, SQ], bf16, tag="qT")
            kT_sb = jpool.tile([D_K, SKV], bf16, tag="kT")
            for t in range(NQC):
                pp = ps_proj.tile([D_K, 512], f32, tag="pqk")
                for i in range(MB):
                    nc.tensor.matmul(
                        pp,
                        lhsT=wq(i),
                        rhs=q_sb[:, i, t * 512 : (t + 1) * 512],
                        start=(i == 0),
                        stop=(i == MB - 1),
                    )
                nc.vector.tensor_copy(qT_sb[:, t * 512 : (t + 1) * 512], pp)
            for t in range(NKC):
                pp = ps_proj.tile([D_K, 512], f32, tag="pqk")
                for i in range(MB):
                    nc.tensor.matmul(
                        pp,
                        lhsT=wk(i),
                        rhs=k_sb[:, i, t * 512 : (t + 1) * 512],
                        start=(i == 0),
                        stop=(i == MB - 1),
                    )
                nc.vector.tensor_copy(kT_sb[:, t * 512 : (t + 1) * 512], pp)

            # ---- v-aug [128, 16, 65] bf16 (col 64 = ones) ----
            vaug = jpool.tile([P, JT, D_K + 1], bf16, tag="vaug")
            nc.vector.memset(vaug[:, :, D_K : D_K + 1], 1.0)

            # ---- E tiles [128, 16, 1024] bf16 ----
            E = jpool.tile([P, JT, SQ], bf16, tag="E")

            # ---- transposed output accumulator [65, 1024] f32 = 2 banks,
            # one accumulation group per bank ----
            oTp = ps_o.tile([D_K + 1, SQ], f32, tag="oT")

            # ---- main pipeline over skv tiles ----
            for j in range(JT):
                g, jj = divmod(j, JPG)
                # v projection for this skv block: [128, 64]
                pv = ps_pv.tile([P, D_K], f32, tag="pv")
                for i in range(MB):
                    nc.tensor.matmul(
                        pv,
                        lhsT=vgs[g][:, jj, i, :],
                        rhs=wv(i),
                        start=(i == 0),
                        stop=(i == MB - 1),
                    )
                nc.vector.tensor_copy(vaug[:, j, 0:D_K], pv)

                # transposed scores [skv_tile 128, sq 1024] (unscaled)
                sp = ps_s.tile([P, SQ], f32, tag="sp")
                for c in range(NQC):
                    nc.tensor.matmul(
                        sp[:, c * 512 : (c + 1) * 512],
                        lhsT=kT_sb[:, j * P : (j + 1) * P],
                        rhs=qT_sb[:, c * 512 : (c + 1) * 512],
                        start=True,
                        stop=True,
                    )
                # E = exp(s / sqrt(d_model)) straight from PSUM, cast bf16
                nc.scalar.activation(
                    out=E[:, j, :],
                    in_=sp,
                    func=mybir.ActivationFunctionType.Exp,
                    scale=float(D_MODEL) ** -0.5,
                )
                # masked positions -> 1.0 (== exp(1e-9) in fp32)
                nc.vector.copy_predicated(
                    out=E[:, j, :], mask=mgs[g][:, jj, :], data=ones_bf
                )
                # PV accumulation (transposed): oT[65, sq] += vaug_j.T @ E_j
                for c in range(NQC):
                    nc.tensor.matmul(
                        oTp[:, c * 512 : (c + 1) * 512],
                        lhsT=vaug[:, j, :],
                        rhs=E[:, j, c * 512 : (c + 1) * 512],
                        start=(j == 0),
                        stop=(j == JT - 1),
                    )

            # ---- finalize: transpose oT back, divide by ones-row ----
            oT_sb = jpool.tile([D_K + 1, SQ], f32, tag="oTs")
            nc.vector.tensor_copy(oT_sb, oTp)
            ob = fpool.tile([P, IT, D_K], f32, tag="ob", bufs=1)
            for i in range(IT):
                tp = ps_s.tile([P, D_K + 1], f32, tag="sp")
                nc.tensor.transpose(
                    tp, in_=oT_sb[:, i * P : (i + 1) * P], identity=idn
                )
                r = fpool.tile([P, 1], f32, tag="r")
                nc.vector.reciprocal(r, tp[:, D_K : D_K + 1])
                nc.vector.tensor_scalar_mul(ob[:, i, :], tp[:, 0:D_K], r)
            nc.sync.dma_start(out=out_d[:], in_=ob)

    nc.finalize()
    return nc


def _get_nc():
    global _cached_nc
    if _cached_nc is None:
        _cached_nc = _build_nc()
    return _cached_nc


def _pack_mb(x_t):
    """[D_MODEL, s] -> [128, MB, s] (m-block packed, contiguous)."""
    s = x_t.shape[1]
    return np.ascontiguousarray(x_t.reshape(MB, P, s).transpose(1, 0, 2))


def _shard_inputs(query, key, value, mask, w_q, w_k, w_v):
    """Host-side shard + layout prep. Core c -> (batch c//2, q-half c%2)."""
    w_all = np.concatenate(
        [
            w.T.astype(_BF16).reshape(MB, P, D_K).transpose(1, 0, 2)
            for w in (w_q, w_k, w_v)
        ],
        axis=2,
    )
    w_all = np.ascontiguousarray(w_all)
    in_maps = []
    for c in range(N_CORES):
        b, h = divmod(c, 2)
        s0 = h * SQ
        q_t = query[b, s0 : s0 + SQ, :].T.astype(_BF16)
        k_t = key[b].T.astype(_BF16)
        v_t = value[b].T.astype(_BF16)
        m_t = mask[b, s0 : s0 + SQ, :].T.astype(np.uint8)
        in_maps.append(
            {
                "w_all": w_all,
                "q_t": _pack_mb(q_t),
                "k_t": _pack_mb(k_t),
                # [m, skv] -> [128, JT, MB, 128]: [p][j][i][s']
                "v_t": np.ascontiguousarray(
                    v_t.reshape(MB, P, JT, P).transpose(1, 2, 0, 3)
                ),
                # [skv, sq] -> [128, JT, SQ]
                "mask_t": np.ascontiguousarray(
                    m_t.reshape(JT, P, SQ).transpose(1, 0, 2)
                ),
            }
        )
    return in_maps


def run(inputs, trace=False):
    """Run the SPMD kernel; returns (output [B,S,D_K] f32, BassKernelResults)."""
    from concourse.bass_utils import run_bass_kernel_spmd

    nc = _get_nc()
    in_maps = _shard_inputs(**inputs)
    res = run_bass_kernel_spmd(
        nc, in_maps, core_ids=list(range(N_CORES)), trace=trace
    )
    out = np.empty((B, S, D_K), np.float32)
    for c in range(N_CORES):
        b, h = divmod(c, 2)
        # device out is [128, IT, 64]: row = i*128+p
        o = res.results[c]["out"].transpose(1, 0, 2).reshape(SQ, D_K)
        out[b, h * SQ : (h + 1) * SQ, :] = o
    return out, res


def kernel(**inputs):
    out, _ = run(inputs, trace=False)
    return out


# revision 8
# speedup vs baseline: 1.3226x; 1.0157x over previous
"""AttentionHead kernel for 8 Trainium2 NeuronCores (SPMD data-parallel).

Problem: q/k/v projections [1024->64] + masked softmax attention,
B=4, S=2048, d_model=1024, d_k=64.

Sharding: 8 cores = 4 batches x 2 query-halves. Each core handles one
(batch, q-half): query shard [1024, 1024], full key/value for its batch
[2048, 1024], mask shard [1024, 2048]. Weights replicated.

Per-core device pipeline (everything contracts on the partition dim;
all inputs host-packed so each DMA is one large contiguous transfer):
  - projections: qT [64, sq], kT [64, skv] via matmul(lhsT=w_t, rhs=xT)
  - v projected per skv-block to natural [128, 64] (lhsT=valueT block),
    augmented with a ones column
  - scores computed TRANSPOSED [skv_tile=128, sq=1024]: no probability
    transpose needed anywhere
  - ACT exp reads scores straight from PSUM with the 1024**-0.5 scale
    fused; masked positions then set to 1.0 (==exp(1e-9) in fp32) via
    copy_predicated on the bf16 E tile
  - PV accumulates transposed: oT[65, sq] += vaug_j.T @ E_j; row 64 is
    the softmax denominator (free via the ones column)
  - finalize: PE-transpose oT back to [sq, 65], multiply by reciprocal
    of column 64, one batched output DMA
"""

import numpy as np
import ml_dtypes

B = 4
S = 2048
D_MODEL = 1024
D_K = 64
N_CORES = 8

P = 128
SQ = S // 2          # per-core query rows (1024)
SKV = S              # per-core kv rows (2048)
MB = D_MODEL // P    # 8 m-blocks (contraction)
JT = SKV // P        # 16 skv tiles
IT = SQ // P         # 8 sq tiles
NQC = SQ // 512      # 2 q chunks
NKC = SKV // 512     # 4 k chunks
VG = 4               # v/mask DMA groups
JPG = JT // VG       # skv tiles per DMA group

_BF16 = ml_dtypes.bfloat16

_cached_nc = None


def _build_nc():
    import concourse.mybir as mybir
    import concourse.tile as tile
    from concourse import bacc
    from concourse.masks import make_identity

    bf16 = mybir.dt.bfloat16
    f32 = mybir.dt.float32
    u8 = mybir.dt.uint8

    nc = bacc.Bacc(None, target_bir_lowering=False)

    w_d = nc.dram_tensor("w_all", [P, MB, 3 * D_K], bf16, kind="ExternalInput")
    q_d = nc.dram_tensor("q_t", [P, MB, SQ], bf16, kind="ExternalInput")
    k_d = nc.dram_tensor("k_t", [P, MB, SKV], bf16, kind="ExternalInput")
    v_d = nc.dram_tensor("v_t", [P, JT, MB, P], bf16, kind="ExternalInput")
    m_d = nc.dram_tensor("mask_t", [P, JT, SQ], u8, kind="ExternalInput")
    out_d = nc.dram_tensor("out", [P, IT, D_K], f32, kind="ExternalOutput")

    with tile.TileContext(nc) as tc:
        with (
            tc.tile_pool(name="const", bufs=1) as cpool,
            tc.tile_pool(name="inp", bufs=1) as ipool,
            tc.tile_pool(name="proj", bufs=1) as jpool,
            tc.tile_pool(name="fin", bufs=2) as fpool,
            tc.tile_pool(name="ps_proj", bufs=1, space="PSUM") as ps_proj,
            tc.tile_pool(name="ps_pv", bufs=1, space="PSUM") as ps_pv,
            tc.tile_pool(name="ps_s", bufs=2, space="PSUM") as ps_s,
            tc.tile_pool(name="ps_o", bufs=1, space="PSUM") as ps_o,
        ):
            # ---- input DMAs: few, large, streamed in compute order ----
            w_sb = cpool.tile([P, MB, 3 * D_K], bf16, tag="w")
            nc.sync.dma_start(out=w_sb, in_=w_d[:])
            q_sb = ipool.tile([P, MB, SQ], bf16, tag="q")
            nc.sync.dma_start(out=q_sb, in_=q_d[:])

            # k in 4 column-chunks so kT projection / scores start early
            kcs = []
            for t in range(NKC):
                kc = ipool.tile([P, MB, 512], bf16, tag=f"k{t}")
                nc.sync.dma_start(out=kc, in_=k_d[:, :, t * 512 : (t + 1) * 512])
                kcs.append(kc)

            # v and mask trickle in groups, interleaved; mask ahead of v
            mgs = []
            vgs = []
            for g in range(VG):
                mg = ipool.tile([P, JPG, SQ], u8, tag=f"m{g}")
                nc.sync.dma_start(out=mg, in_=m_d[:, g * JPG : (g + 1) * JPG, :])
                mgs.append(mg)
                vg = ipool.tile([P, JPG, MB, P], bf16, tag=f"v{g}")
                nc.sync.dma_start(out=vg, in_=v_d[:, g * JPG : (g + 1) * JPG, :, :])
                vgs.append(vg)

            def wq(i):
                return w_sb[:, i, 0:D_K]

            def wk(i):
                return w_sb[:, i, D_K : 2 * D_K]

            def wv(i):
                return w_sb[:, i, 2 * D_K : 3 * D_K]

            # ---- constants ----
            ones_bf = cpool.tile([P, SQ], bf16, tag="ones")
            nc.vector.memset(ones_bf, 1.0)
            idn = cpool.tile([D_K + 1, D_K + 1], f32, tag="idn")
            make_identity(nc, idn)

            # ---- q/k projections, duplicated onto both partition halves
            # (qTd/kTd rows 0-63 == rows 64-127) so scores can run as two
            # concurrent row-group matmuls ----
            qTd = jpool.tile([P, SQ], bf16, tag="qT")
            kTd = jpool.tile([P, SKV], bf16, tag="kT")
            for t in range(NQC):
                pp = ps_proj.tile([D_K, 512], f32, tag="pqk")
                for i in range(MB):
                    nc.tensor.matmul(
                        pp,
                        lhsT=wq(i),
                        rhs=q_sb[:, i, t * 512 : (t + 1) * 512],
                        start=(i == 0),
                        stop=(i == MB - 1),
                    )
                sl = slice(t * 512, (t + 1) * 512)
                nc.scalar.copy(qTd[0:D_K, sl], pp)
                nc.scalar.copy(qTd[D_K:P, sl], pp)
            for t in range(NKC):
                pp = ps_proj.tile([D_K, 512], f32, tag="pqk")
                for i in range(MB):
                    nc.tensor.matmul(
                        pp,
                        lhsT=wk(i),
                        rhs=kcs[t][:, i, :],
                        start=(i == 0),
                        stop=(i == MB - 1),
                    )
                sl = slice(t * 512, (t + 1) * 512)
                nc.scalar.copy(kTd[0:D_K, sl], pp)
                nc.scalar.copy(kTd[D_K:P, sl], pp)

            # ---- v-aug [128, 16, 65] bf16 (col 64 = ones) ----
            vaug = jpool.tile([P, JT, D_K + 1], bf16, tag="vaug")
            nc.vector.memset(vaug[:, :, D_K : D_K + 1], 1.0)

            # ---- E tiles [128, 16, 1024] bf16 ----
            E = jpool.tile([P, JT, SQ], bf16, tag="E")

            # ---- transposed output accumulator [65, 1024] f32 = 2 banks,
            # one accumulation group per bank ----
            oTp = ps_o.tile([D_K + 1, SQ], f32, tag="oT")

            # ---- main pipeline over skv tiles ----
            for j in range(JT):
                g, jj = divmod(j, JPG)
                # v projection for this skv block: [128, 64]
                pv = ps_pv.tile([P, D_K], f32, tag="pv")
                for i in range(MB):
                    nc.tensor.matmul(
                        pv,
                        lhsT=vgs[g][:, jj, i, :],
                        rhs=wv(i),
                        start=(i == 0),
                        stop=(i == MB - 1),
                    )
                nc.vector.tensor_copy(vaug[:, j, 0:D_K], pv)

                # transposed scores [skv_tile 128, sq 1024] (unscaled):
                # two concurrent K=64 matmuls in PE row-groups 0 and 64
                sp = ps_s.tile([P, SQ], f32, tag="sp")
                jsl = slice(j * P, (j + 1) * P)
                nc.tensor.matmul(
                    sp[:, 0:512],
                    lhsT=kTd[0:D_K, jsl],
                    rhs=qTd[0:D_K, 0:512],
                    start=True,
                    stop=True,
                    tile_position=(0, 0),
                )
                nc.tensor.matmul(
                    sp[:, 512:1024],
                    lhsT=kTd[D_K:P, jsl],
                    rhs=qTd[D_K:P, 512:1024],
                    start=True,
                    stop=True,
                    tile_position=(64, 0),
                )
                # E = exp(s / sqrt(d_model)) straight from PSUM, cast bf16
                nc.scalar.activation(
                    out=E[:, j, :],
                    in_=sp,
                    func=mybir.ActivationFunctionType.Exp,
                    scale=float(D_MODEL) ** -0.5,
                )
                # masked positions -> 1.0 (== exp(1e-9) in fp32)
                nc.vector.copy_predicated(
                    out=E[:, j, :], mask=mgs[g][:, jj, :], data=ones_bf
                )
                # PV accumulation (transposed): oT[65, sq] += vaug_j.T @ E_j
                for c in range(NQC):
                    nc.tensor.matmul(
                        oTp[:, c * 512 : (c + 1) * 512],
                        lhsT=vaug[:, j, :],
                        rhs=E[:, j, c * 512 : (c + 1) * 512],
                        start=(j == 0),
                        stop=(j == JT - 1),
                    )

            # ---- finalize: transpose oT back, divide by ones-row ----
            oT_sb = jpool.tile([D_K + 1, SQ], f32, tag="oTs")
            nc.vector.tensor_copy(oT_sb, oTp)
            ob = fpool.tile([P, IT, D_K], f32, tag="ob", bufs=1)
            for i in range(IT):
                tp = ps_s.tile([P, D_K + 1], f32, tag="sp")
                nc.tensor.transpose(
                    tp, in_=oT_sb[:, i * P : (i + 1) * P], identity=idn
                )
                r = fpool.tile([P, 1], f32, tag="r")
                nc.vector.reciprocal(r, tp[:, D_K : D_K + 1])
                nc.vector.tensor_scalar_mul(ob[:, i, :], tp[:, 0:D_K], r)
            nc.sync.dma_start(out=out_d[:], in_=ob)

    nc.finalize()
    return nc


def _get_nc():
    global _cached_nc
    if _cached_nc is None:
        _cached_nc = _build_nc()
    return _cached_nc


def _pack_mb(x_t):
    """[D_MODEL, s] -> [128, MB, s] (m-block packed, contiguous)."""
    s = x_t.shape[1]
    return np.ascontiguousarray(x_t.reshape(MB, P, s).transpose(1, 0, 2))


def _shard_inputs(query, key, value, mask, w_q, w_k, w_v):
    """Host-side shard + layout prep. Core c -> (batch c//2, q-half c%2)."""
    w_all = np.concatenate(
        [
            w.T.astype(_BF16).reshape(MB, P, D_K).transpose(1, 0, 2)
            for w in (w_q, w_k, w_v)
        ],
        axis=2,
    )
    w_all = np.ascontiguousarray(w_all)
    in_maps = []
    for c in range(N_CORES):
        b, h = divmod(c, 2)
        s0 = h * SQ
        q_t = query[b, s0 : s0 + SQ, :].T.astype(_BF16)
        k_t = key[b].T.astype(_BF16)
        v_t = value[b].T.astype(_BF16)
        m_t = mask[b, s0 : s0 + SQ, :].T.astype(np.uint8)
        in_maps.append(
            {
                "w_all": w_all,
                "q_t": _pack_mb(q_t),
                "k_t": _pack_mb(k_t),
                # [m, skv] -> [128, JT, MB, 128]: [p][j][i][s']
                "v_t": np.ascontiguousarray(
                    v_t.reshape(MB, P, JT, P).transpose(1, 2, 0, 3)
                ),
                # [skv, sq] -> [128, JT, SQ]
                "mask_t": np.ascontiguousarray(
                    m_t.reshape(JT, P, SQ).transpose(1, 0, 2)
                ),
            }
        )
    return in_maps


def run(inputs, trace=False):
    """Run the SPMD kernel; returns (output [B,S,D_K] f32, BassKernelResults)."""
    from concourse.bass_utils import run_bass_kernel_spmd

    nc = _get_nc()
    in_maps = _shard_inputs(**inputs)
    res = run_bass_kernel_spmd(
        nc, in_maps, core_ids=list(range(N_CORES)), trace=trace
    )
    out = np.empty((B, S, D_K), np.float32)
    for c in range(N_CORES):
        b, h = divmod(c, 2)
        # device out is [128, IT, 64]: row = i*128+p
        o = res.results[c]["out"].transpose(1, 0, 2).reshape(SQ, D_K)
        out[b, h * SQ : (h + 1) * SQ, :] = o
    return out, res


def kernel(**inputs):
    out, _ = run(inputs, trace=False)
    return out
